# revision 15
# baseline (speedup 1.0000x reference)
"""Trainium2 Bass kernel for y = x @ W.T + b  (x: [16384,1024] f32,
W: [1024,1024] f32, b: [1024] f32) on 8 NeuronCores.

Data-parallel: x is split along batch into 8 shards of 2048 rows;
W and b are replicated. Each core computes its y shard with bf16
matmuls accumulating in fp32 PSUM; bias is fused into the PSUM->SBUF
eviction on the Scalar engine, which downcasts to bf16 (host upcasts
back to f32 -- halves store traffic; adds ~3e-3 rel err, well under
the gate). Host-side we pre-transpose x (and W) to put the contraction
dim on SBUF partitions, so no on-chip transposes are needed, and group
DRAM layouts so every DMA is 128 long contiguous runs.

Schedule per core (bq = one of 4 batch chunks of 512 rows):
- All input DMAs go on the Sync HWDGE ring in consumption order
  ((w[ko], x[bq0,ko]) pairs, bias, then x[bq1..3]); the first pair
  lands ~3 us after the body starts. N=128 dummy matmuls bridge the
  PE clock-gate window (HAM needs ~3.4-6.8 us of *continuous* busy to
  unthrottle 1.2 -> 2.4 GHz) until the first real matmul, which then
  runs cold but useful until HAM fires.
- bq0 runs contraction-outer across all 8 PSUM banks, consuming the
  pair stream as it arrives; bq1..3 run output-tile-outer (one PSUM
  bank at a time).
- Each output tile is evicted on the Scalar engine (bias fused,
  bf16 downcast) and stored immediately (128 KiB bf16 stores on the
  Sync ring, gated on the eviction counter), so output DMA spreads
  across the whole kernel. The very last tile is split into two N=256
  halves, evicted on the otherwise-idle Vector engine, stores split
  across the Sync/Scalar rings, so the kernel tail is one 32 KiB
  store + HBM write receipt.
"""

import sys

if "/opt/trn_rl_repo" not in sys.path:
    sys.path.insert(0, "/opt/trn_rl_repo")

import ml_dtypes
import numpy as np

# concourse's trace path imports antenv.axon_hooks, which this image lacks.
# Register a working NTFF-profile hook (via the axon PJRT .so) so tracing
# works when requested, degrading to no-op if anything is missing.
try:
    import antenv.axon_hooks  # noqa: F401
except ImportError:
    import types as _types

    def _make_hook():
        try:
            from trn_agent_boot.trn_boot import _ntff_profile_via_ctypes

            return _ntff_profile_via_ctypes("/opt/axon/libaxon_pjrt.so")
        except Exception:
            return None

    _hooks = _types.ModuleType("antenv.axon_hooks")
    _hooks.get_axon_ntff_profile_hook = _make_hook
    _hooks.set_axon_ntff_profile_hook = lambda h: None
    sys.modules["antenv.axon_hooks"] = _hooks

BATCH = 16384
IN_F = 1024
OUT_F = 1024
NCORES = 8
P = 128
KO = IN_F // P  # 8 contraction tiles
MO = OUT_F // P  # 8 output-feature tiles
BS = BATCH // NCORES  # 2048 rows per core
FD = 512  # matmul moving free dim (one PSUM bank of fp32)
NB = BS // FD  # 4 batch chunks per core
# N=128 dummy matmuls bridging until the first inputs land (~1.4 us).
# Real matmuls run cold-but-useful after that (PE clock ramps at the
# HAM window regardless of what it executes), so fewer is better.
N_WARM = 31

_cache = {}
LAST_RESULT = None


def _build():
    import concourse.mybir as mybir
    import concourse.tile as tile
    from concourse import bacc

    nc = bacc.Bacc(None, target_bir_lowering=False)
    # xT4[p, bq, ko, fd] = x[bq*FD + fd, ko*P + p]
    xT = nc.declare_dram_parameter(
        "xT", [P, NB, KO, FD], mybir.dt.bfloat16, isOutput=False
    )
    # w3[p, ko, mo, c] = W[mo*P + c, ko*P + p]  (ko-major: bq0 consumes
    # weights one ko chunk at a time). NOTE: w and x must stay SEPARATE
    # SBUF tiles -- packing them into one tile puts the PE's two SBUF
    # read ports (weights + moving operand) on the same sub-banks and
    # slows every matmul ~20%.
    w3 = nc.declare_dram_parameter(
        "w3", [P, KO, MO, P], mybir.dt.bfloat16, isOutput=False
    )
    bias = nc.declare_dram_parameter("bias", [P, MO], mybir.dt.float32, isOutput=False)
    # out4[p, bq, mo, fd] = y[bq*FD + fd, mo*P + p]
    out = nc.declare_dram_parameter(
        "out", [P, NB, MO, FD], mybir.dt.bfloat16, isOutput=True
    )

    with tile.TileContext(nc) as tc:
        with (
            tc.tile_pool(name="const", bufs=1) as cpool,
            tc.tile_pool(name="outp", bufs=3) as opool,
            tc.tile_pool(name="psum", bufs=8, space="PSUM") as ppool,
        ):
            x_sb = cpool.tile([P, NB, KO, FD], mybir.dt.bfloat16)
            w_sb = cpool.tile([P, KO, MO, P], mybir.dt.bfloat16)
            b_sb = cpool.tile([P, MO], mybir.dt.float32)
            wu_sb = cpool.tile([P, P], mybir.dt.bfloat16)
            # memset on GpSimd: it has no other body work, so the first
            # warm-up matmul isn't queued behind Vector's preamble.
            nc.gpsimd.memset(wu_sb[:], 0.0)
            # All inputs on ONE HWDGE ring (sync), in consumption order.
            # A single ring streams (w, x) ko-pairs at ~1.35 us cadence;
            # phase A consumes a chunk in 1.73 us (2.4 GHz warm) or more,
            # so the stream stays ahead with no stalls. Splitting w onto a
            # second ring was measured WORSE: cross-ring SDMA interleaving
            # slowed the w stream enough to stall phase A ~4 us waiting on
            # late w chunks, and the first pair landed no earlier. FIFO
            # order also guarantees arrival order == consumption order.
            # NOTE: splitting w0 into halves (to start the first matmuls
            # ~0.3 us earlier) was measured WORSE: the extra issue slot
            # delays later pair chunks, phase A's thin DMA margin
            # (~0.35 us/chunk at 2.4 GHz) flips into a multi-us stall,
            # and that stall re-throttles the PE clock (HAM MID window).
            for ko in range(KO):
                nc.sync.dma_start(w_sb[:, ko], w3[:, ko])
                nc.sync.dma_start(x_sb[:, 0, ko], xT[:, 0, ko])
            nc.sync.dma_start(b_sb[:], bias[:])
            # Bulk x in half-chunks: phase B consumes ko-ascending, so
            # landing x1[ko0-3] ~3 us earlier removes a ~1 us stall at
            # the phase A -> B boundary (measured: phase B's first MM
            # waited on the full-1-MiB x1 transfer).
            for bq in range(1, NB):
                h = KO // 2
                nc.sync.dma_start(x_sb[:, bq, 0:h], xT[:, bq, 0:h])
                nc.sync.dma_start(x_sb[:, bq, h:KO], xT[:, bq, h:KO])

            # PE HAM warm-up: keep the PE continuously busy from body
            # start until the first input chunks land (~3 us), so the
            # clock-gate window starts counting as early as possible.
            wu_ps = ppool.tile([P, FD], mybir.dt.float32, tag="ps")
            for _ in range(N_WARM):
                nc.tensor.matmul(
                    wu_ps[:, :P], wu_sb[:], wu_sb[:], start=True, stop=True
                )

            # bq0: contraction-outer over all 8 PSUM banks, consuming
            # (w, x) ko-chunks in DMA arrival order.
            ps0 = [
                ppool.tile([P, FD], mybir.dt.float32, tag="ps", name=f"ps0_{mo}")
                for mo in range(MO)
            ]
            o_sb = opool.tile([P, MO, FD], mybir.dt.bfloat16)
            for ko in range(KO):
                for mo in range(MO):
                    nc.tensor.matmul(
                        ps0[mo][:],
                        w_sb[:, ko, mo],
                        x_sb[:, 0, ko],
                        start=(ko == 0),
                        stop=(ko == KO - 1),
                    )
            # Evictions alternate Vector/Scalar so the 8-bank burst at the
            # end of bq0 drains in two parallel ~2.4 us chains instead of
            # one ~5.5 us chain (whose backlog stalled later bank reuse).
            for mo in range(MO):
                if mo % 2 == 0:
                    nc.vector.tensor_scalar_add(
                        o_sb[:, mo], ps0[mo][:], b_sb[:, mo : mo + 1]
                    )
                else:
                    nc.scalar.activation(
                        o_sb[:, mo],
                        ps0[mo][:],
                        mybir.ActivationFunctionType.Identity,
                        bias=b_sb[:, mo : mo + 1],
                    )
                nc.sync.dma_start(out[:, 0, mo], o_sb[:, mo])

            # bq1..3: output-tile-outer, one PSUM bank at a time;
            # evict + store each tile as soon as it completes.
            for bq in range(1, NB):
                o_sb = opool.tile([P, MO, FD], mybir.dt.bfloat16)
                for mo in range(MO):
                    if bq == NB - 1 and mo == MO - 1:
                        # Very last tile: split 256+128+128 so the final
                        # dependency chain (evict -> 128-descriptor store
                        # issue -> HBM receipt) hangs off a narrow N=128
                        # piece. Evicts and stores alternate engines/rings
                        # so the three chains overlap.
                        pieces = [(0, 256), (256, 128), (384, 128)]
                        ev_eng = ["scalar", "vector", "vector"]
                        st_eng = [nc.sync, nc.scalar, nc.sync]
                        for h, (off, width) in enumerate(pieces):
                            hs = slice(off, off + width)
                            ps = ppool.tile(
                                [P, FD], mybir.dt.float32, tag="ps", name=f"ps_l{h}"
                            )
                            for ko in range(KO):
                                nc.tensor.matmul(
                                    ps[:, :width],
                                    w_sb[:, ko, mo],
                                    x_sb[:, bq, ko, hs],
                                    start=(ko == 0),
                                    stop=(ko == KO - 1),
                                )
                            if ev_eng[h] == "scalar":
                                nc.scalar.activation(
                                    o_sb[:, mo, hs],
                                    ps[:, :width],
                                    mybir.ActivationFunctionType.Identity,
                                    bias=b_sb[:, mo : mo + 1],
                                )
                            else:
                                nc.vector.tensor_scalar_add(
                                    o_sb[:, mo, hs], ps[:, :width], b_sb[:, mo : mo + 1]
                                )
                            st_eng[h].dma_start(out[:, bq, mo, hs], o_sb[:, mo, hs])
                    else:
                        ps = ppool.tile([P, FD], mybir.dt.float32, tag="ps")
                        for ko in range(KO):
                            nc.tensor.matmul(
                                ps[:],
                                w_sb[:, ko, mo],
                                x_sb[:, bq, ko],
                                start=(ko == 0),
                                stop=(ko == KO - 1),
                            )
                        if mo % 2 == 0:
                            nc.vector.tensor_scalar_add(
                                o_sb[:, mo], ps[:], b_sb[:, mo : mo + 1]
                            )
                        else:
                            nc.scalar.activation(
                                o_sb[:, mo],
                                ps[:],
                                mybir.ActivationFunctionType.Identity,
                                bias=b_sb[:, mo : mo + 1],
                            )
                        nc.sync.dma_start(out[:, bq, mo], o_sb[:, mo])

    nc.compile()
    return nc


def kernel(x, weight, bias):
    global LAST_RESULT
    from concourse.bass_utils import run_bass_kernel_spmd

    if "nc" not in _cache:
        _cache["nc"] = _build()
    nc = _cache["nc"]

    x = np.asarray(x, dtype=np.float32)
    weight = np.asarray(weight, dtype=np.float32)
    bias = np.asarray(bias, dtype=np.float32)

    bf16 = ml_dtypes.bfloat16
    # w3[p, ko, mo, c] = W[mo*P + c, ko*P + p]
    wb = weight.astype(bf16).reshape(MO, P, KO, P)  # [mo, c, ko, p]
    w3 = np.ascontiguousarray(wb.transpose(3, 2, 0, 1))  # [p, ko, mo, c]
    # bias laid out [P, MO]: b[p, mo] = bias[mo*P + p]
    b_t = np.ascontiguousarray(bias.astype(np.float32).reshape(MO, P).T)

    in_maps = []
    for c in range(NCORES):
        xs = x[c * BS : (c + 1) * BS].astype(bf16)
        # xT4[p, bq, ko, fd] = x[bq*FD + fd, ko*P + p]
        xr = xs.reshape(NB, FD, KO, P)  # [bq, fd, ko, p]
        xT = np.ascontiguousarray(xr.transpose(3, 0, 2, 1))  # [p, bq, ko, fd]
        in_maps.append({"xT": xT, "w3": w3, "bias": b_t})

    res = run_bass_kernel_spmd(nc, in_maps, list(range(NCORES)))
    LAST_RESULT = res

    y = np.empty((BATCH, OUT_F), dtype=np.float32)
    for c in range(NCORES):
        o = res.results[c]["out"]  # [p, bq, mo, fd] bf16
        y[c * BS : (c + 1) * BS] = (
            o.astype(np.float32).transpose(1, 3, 2, 0).reshape(BS, OUT_F)
        )
    return y



# revision 18
# speedup vs baseline: 1.1683x; 1.1683x over previous
"""Trainium2 Bass kernel for y = x @ W.T + b  (x: [16384,1024] f32,
W: [1024,1024] f32, b: [1024] f32) on 8 NeuronCores.

Data-parallel: x is split along batch into 8 shards of 2048 rows;
W and b are replicated. Each core computes its y shard with bf16
matmuls accumulating in fp32 PSUM; bias is fused into the PSUM->SBUF
eviction on the Scalar engine, which downcasts to bf16 (host upcasts
back to f32 -- halves store traffic; adds ~3e-3 rel err, well under
the gate). Host-side we pre-transpose x (and W) to put the contraction
dim on SBUF partitions, so no on-chip transposes are needed, and group
DRAM layouts so every DMA is 128 long contiguous runs.

Schedule per core (bq = one of 4 batch chunks of 512 rows):
- All input DMAs go on the Sync HWDGE ring in consumption order:
  (w[ko], x[bq0,ko]) pairs, bias, then x[bq1..3] in half-chunks. The
  first pair lands ~3.3 us after the body starts; pair cadence
  (~1.35 us) stays ahead of phase-A consumption (1.73 us/chunk warm).
- N=128 dummy matmuls bridge from body start until the first pair
  lands. This must be GAP-FREE: a >~1.5 us PE idle gap resets the HAM
  clock-gate ramp (measured: a 2.4 us gap delayed full clock by 5 us,
  and a mid-stream DMA stall re-throttled an already-warm PE). The PE
  warm clock itself varies with the chassis power state: measured
  2.4 GHz (216 ns per N=512 bf16 matmul) in one session and 2.0 GHz
  (259 ns) in another; cold is half that. The matmul stream itself is
  at the issue floor in both states -- per-MM overhead is ~0 (verified
  by experiments: weight reuse, LDWEIGHTS dedup, and per-MM semaphore
  stripping all left the cadence unchanged).
- bq0 runs contraction-outer across all 8 PSUM banks, consuming the
  pair stream as it arrives; bq1..3 run output-tile-outer (one PSUM
  bank at a time).
- Output tiles are evicted (bias fused, bf16 downcast) alternating
  between the Vector and Scalar engines, and stored on the ring
  opposite the evict engine, so no queue does evict+store
  back-to-back and the 8-bank burst at the end of bq0 drains in two
  parallel chains. The very last tile is split 256+128+128 so the
  final chain (evict -> 128-descriptor store issue -> HBM receipt)
  hangs off a narrow N=128 piece.
"""

import sys

if "/opt/trn_rl_repo" not in sys.path:
    sys.path.insert(0, "/opt/trn_rl_repo")

import ml_dtypes
import numpy as np

# concourse's trace path imports antenv.axon_hooks, which this image lacks.
# Register a working NTFF-profile hook (via the axon PJRT .so) so tracing
# works when requested, degrading to no-op if anything is missing.
try:
    import antenv.axon_hooks  # noqa: F401
except ImportError:
    import types as _types

    def _make_hook():
        try:
            from trn_agent_boot.trn_boot import _ntff_profile_via_ctypes

            return _ntff_profile_via_ctypes("/opt/axon/libaxon_pjrt.so")
        except Exception:
            return None

    _hooks = _types.ModuleType("antenv.axon_hooks")
    _hooks.get_axon_ntff_profile_hook = _make_hook
    _hooks.set_axon_ntff_profile_hook = lambda h: None
    sys.modules["antenv.axon_hooks"] = _hooks

BATCH = 16384
IN_F = 1024
OUT_F = 1024
NCORES = 8
P = 128
KO = IN_F // P  # 8 contraction tiles
MO = OUT_F // P  # 8 output-feature tiles
BS = BATCH // NCORES  # 2048 rows per core
FD = 512  # matmul moving free dim (one PSUM bank of fp32)
NB = BS // FD  # 4 batch chunks per core
# N=128 dummy matmuls bridging until the first inputs land (~1.4 us).
# Real matmuls run cold-but-useful after that (PE clock ramps at the
# HAM window regardless of what it executes), so fewer is better.
N_WARM = 31

_cache = {}
LAST_RESULT = None


def _build():
    import concourse.mybir as mybir
    import concourse.tile as tile
    from concourse import bacc

    nc = bacc.Bacc(None, target_bir_lowering=False)
    # xT4[p, bq, ko, fd] = x[bq*FD + fd, ko*P + p]
    xT = nc.declare_dram_parameter(
        "xT", [P, NB, KO, FD], mybir.dt.bfloat16, isOutput=False
    )
    # w3[p, ko, mo, c] = W[mo*P + c, ko*P + p]  (ko-major: bq0 consumes
    # weights one ko chunk at a time). NOTE: w and x must stay SEPARATE
    # SBUF tiles -- packing them into one tile puts the PE's two SBUF
    # read ports (weights + moving operand) on the same sub-banks and
    # slows every matmul ~20%.
    w3 = nc.declare_dram_parameter(
        "w3", [P, KO, MO, P], mybir.dt.bfloat16, isOutput=False
    )
    bias = nc.declare_dram_parameter("bias", [P, MO], mybir.dt.float32, isOutput=False)
    # out4[p, bq, mo, fd] = y[bq*FD + fd, mo*P + p]
    out = nc.declare_dram_parameter(
        "out", [P, NB, MO, FD], mybir.dt.bfloat16, isOutput=True
    )

    with tile.TileContext(nc) as tc:
        with (
            tc.tile_pool(name="const", bufs=1) as cpool,
            tc.tile_pool(name="outp", bufs=3) as opool,
            tc.tile_pool(name="psum", bufs=8, space="PSUM") as ppool,
        ):
            x_sb = cpool.tile([P, NB, KO, FD], mybir.dt.bfloat16)
            w_sb = cpool.tile([P, KO, MO, P], mybir.dt.bfloat16)
            b_sb = cpool.tile([P, MO], mybir.dt.float32)
            wu_sb = cpool.tile([P, P], mybir.dt.bfloat16)
            # memset on GpSimd: it has no other body work, so the first
            # warm-up matmul isn't queued behind Vector's preamble.
            nc.gpsimd.memset(wu_sb[:], 0.0)
            # All inputs on ONE HWDGE ring (sync), in consumption order.
            # A single ring streams (w, x) ko-pairs at ~1.35 us cadence;
            # phase A consumes a chunk in 1.73 us (2.4 GHz warm) or more,
            # so the stream stays ahead with no stalls. Splitting w onto a
            # second ring was measured WORSE: cross-ring SDMA interleaving
            # slowed the w stream enough to stall phase A ~4 us waiting on
            # late w chunks, and the first pair landed no earlier. FIFO
            # order also guarantees arrival order == consumption order.
            # NOTE: splitting w0 into halves (to start the first matmuls
            # ~0.3 us earlier) was measured WORSE: the extra issue slot
            # delays later pair chunks, phase A's thin DMA margin
            # (~0.35 us/chunk at 2.4 GHz) flips into a multi-us stall,
            # and that stall re-throttles the PE clock (HAM MID window).
            for ko in range(KO):
                nc.sync.dma_start(w_sb[:, ko], w3[:, ko])
                nc.sync.dma_start(x_sb[:, 0, ko], xT[:, 0, ko])
            nc.sync.dma_start(b_sb[:], bias[:])
            # Bulk x in half-chunks: phase B consumes ko-ascending, so
            # landing x1[ko0-3] ~3 us earlier removes a ~1 us stall at
            # the phase A -> B boundary (measured: phase B's first MM
            # waited on the full-1-MiB x1 transfer).
            for bq in range(1, NB):
                h = KO // 2
                nc.sync.dma_start(x_sb[:, bq, 0:h], xT[:, bq, 0:h])
                nc.sync.dma_start(x_sb[:, bq, h:KO], xT[:, bq, h:KO])

            # PE HAM warm-up: keep the PE continuously busy from body
            # start until the first input chunks land (~3 us), so the
            # clock-gate window starts counting as early as possible.
            wu_ps = ppool.tile([P, FD], mybir.dt.float32, tag="ps")
            for _ in range(N_WARM):
                nc.tensor.matmul(
                    wu_ps[:, :P], wu_sb[:], wu_sb[:], start=True, stop=True
                )

            # bq0: contraction-outer over all 8 PSUM banks, consuming
            # (w, x) ko-chunks in DMA arrival order.
            ps0 = [
                ppool.tile([P, FD], mybir.dt.float32, tag="ps", name=f"ps0_{mo}")
                for mo in range(MO)
            ]
            o_sb = opool.tile([P, MO, FD], mybir.dt.bfloat16)
            for ko in range(KO):
                for mo in range(MO):
                    nc.tensor.matmul(
                        ps0[mo][:],
                        w_sb[:, ko, mo],
                        x_sb[:, 0, ko],
                        start=(ko == 0),
                        stop=(ko == KO - 1),
                    )
            # Evictions alternate Vector/Scalar so the 8-bank burst at the
            # end of bq0 drains in two parallel ~2.4 us chains instead of
            # one ~5.5 us chain (whose backlog stalled later bank reuse).
            for mo in range(MO):
                if mo % 2 == 0:
                    nc.vector.tensor_scalar_add(
                        o_sb[:, mo], ps0[mo][:], b_sb[:, mo : mo + 1]
                    )
                else:
                    nc.scalar.activation(
                        o_sb[:, mo],
                        ps0[mo][:],
                        mybir.ActivationFunctionType.Identity,
                        bias=b_sb[:, mo : mo + 1],
                    )
                # Stores alternate rings, opposite to the evict engine,
                # so neither queue does evict+store back-to-back.
                st = nc.scalar if mo % 2 == 0 else nc.sync
                st.dma_start(out[:, 0, mo], o_sb[:, mo])

            # bq1..3: output-tile-outer, one PSUM bank at a time;
            # evict + store each tile as soon as it completes.
            for bq in range(1, NB):
                o_sb = opool.tile([P, MO, FD], mybir.dt.bfloat16)
                for mo in range(MO):
                    if bq == NB - 1 and mo == MO - 1:
                        # Very last tile: split 256+128+128 so the final
                        # dependency chain (evict -> 128-descriptor store
                        # issue -> HBM receipt) hangs off a narrow N=128
                        # piece. Evicts and stores alternate engines/rings
                        # so the three chains overlap.
                        pieces = [(0, 256), (256, 128), (384, 128)]
                        ev_eng = ["scalar", "vector", "vector"]
                        st_eng = [nc.sync, nc.scalar, nc.sync]
                        for h, (off, width) in enumerate(pieces):
                            hs = slice(off, off + width)
                            ps = ppool.tile(
                                [P, FD], mybir.dt.float32, tag="ps", name=f"ps_l{h}"
                            )
                            for ko in range(KO):
                                nc.tensor.matmul(
                                    ps[:, :width],
                                    w_sb[:, ko, mo],
                                    x_sb[:, bq, ko, hs],
                                    start=(ko == 0),
                                    stop=(ko == KO - 1),
                                )
                            if ev_eng[h] == "scalar":
                                nc.scalar.activation(
                                    o_sb[:, mo, hs],
                                    ps[:, :width],
                                    mybir.ActivationFunctionType.Identity,
                                    bias=b_sb[:, mo : mo + 1],
                                )
                            else:
                                nc.vector.tensor_scalar_add(
                                    o_sb[:, mo, hs], ps[:, :width], b_sb[:, mo : mo + 1]
                                )
                            st_eng[h].dma_start(out[:, bq, mo, hs], o_sb[:, mo, hs])
                    else:
                        ps = ppool.tile([P, FD], mybir.dt.float32, tag="ps")
                        for ko in range(KO):
                            nc.tensor.matmul(
                                ps[:],
                                w_sb[:, ko, mo],
                                x_sb[:, bq, ko],
                                start=(ko == 0),
                                stop=(ko == KO - 1),
                            )
                        if mo % 2 == 0:
                            nc.vector.tensor_scalar_add(
                                o_sb[:, mo], ps[:], b_sb[:, mo : mo + 1]
                            )
                        else:
                            nc.scalar.activation(
                                o_sb[:, mo],
                                ps[:],
                                mybir.ActivationFunctionType.Identity,
                                bias=b_sb[:, mo : mo + 1],
                            )
                        st = nc.scalar if mo % 2 == 0 else nc.sync
                        st.dma_start(out[:, bq, mo], o_sb[:, mo])

    nc.compile()
    return nc


def kernel(x, weight, bias):
    global LAST_RESULT
    from concourse.bass_utils import run_bass_kernel_spmd

    if "nc" not in _cache:
        _cache["nc"] = _build()
    nc = _cache["nc"]

    x = np.asarray(x, dtype=np.float32)
    weight = np.asarray(weight, dtype=np.float32)
    bias = np.asarray(bias, dtype=np.float32)

    bf16 = ml_dtypes.bfloat16
    # w3[p, ko, mo, c] = W[mo*P + c, ko*P + p]
    wb = weight.astype(bf16).reshape(MO, P, KO, P)  # [mo, c, ko, p]
    w3 = np.ascontiguousarray(wb.transpose(3, 2, 0, 1))  # [p, ko, mo, c]
    # bias laid out [P, MO]: b[p, mo] = bias[mo*P + p]
    b_t = np.ascontiguousarray(bias.astype(np.float32).reshape(MO, P).T)

    in_maps = []
    for c in range(NCORES):
        xs = x[c * BS : (c + 1) * BS].astype(bf16)
        # xT4[p, bq, ko, fd] = x[bq*FD + fd, ko*P + p]
        xr = xs.reshape(NB, FD, KO, P)  # [bq, fd, ko, p]
        xT = np.ascontiguousarray(xr.transpose(3, 0, 2, 1))  # [p, bq, ko, fd]
        in_maps.append({"xT": xT, "w3": w3, "bias": b_t})

    res = run_bass_kernel_spmd(nc, in_maps, list(range(NCORES)))
    LAST_RESULT = res

    y = np.empty((BATCH, OUT_F), dtype=np.float32)
    for c in range(NCORES):
        o = res.results[c]["out"]  # [p, bq, mo, fd] bf16
        y[c * BS : (c + 1) * BS] = (
            o.astype(np.float32).transpose(1, 3, 2, 0).reshape(BS, OUT_F)
        )
    return y



# revision 19
# speedup vs baseline: 1.1729x; 1.0039x over previous
"""Trainium2 Bass kernel for y = x @ W.T + b  (x: [16384,1024] f32,
W: [1024,1024] f32, b: [1024] f32) on 8 NeuronCores.

Data-parallel: x is split along batch into 8 shards of 2048 rows;
W and b are replicated. Each core computes its y shard with bf16
matmuls accumulating in fp32 PSUM; bias is fused into the PSUM->SBUF
eviction on the Scalar engine, which downcasts to bf16 (host upcasts
back to f32 -- halves store traffic; adds ~3e-3 rel err, well under
the gate). Host-side we pre-transpose x (and W) to put the contraction
dim on SBUF partitions, so no on-chip transposes are needed, and group
DRAM layouts so every DMA is 128 long contiguous runs.

Schedule per core (bq = one of 4 batch chunks of 512 rows):
- All input DMAs go on the Sync HWDGE ring in consumption order:
  (w[ko], x[bq0,ko]) pairs, bias, then x[bq1..3] in half-chunks. The
  first pair lands ~3.3 us after the body starts; pair cadence
  (~1.35 us) stays ahead of phase-A consumption (1.73 us/chunk warm).
- N=128 dummy matmuls bridge from body start until the first pair
  lands. This must be GAP-FREE: a >~1.5 us PE idle gap resets the HAM
  clock-gate ramp (measured: a 2.4 us gap delayed full clock by 5 us,
  and a mid-stream DMA stall re-throttled an already-warm PE). The PE
  warm clock itself varies with the chassis power state: measured
  2.4 GHz (216 ns per N=512 bf16 matmul) in one session and 2.0 GHz
  (259 ns) in another; cold is half that. The matmul stream itself is
  at the issue floor in both states -- per-MM overhead is ~0 (verified
  by experiments: weight reuse, LDWEIGHTS dedup, and per-MM semaphore
  stripping all left the cadence unchanged).
- bq0 runs contraction-outer across all 8 PSUM banks, consuming the
  pair stream as it arrives; bq1..3 run output-tile-outer (one PSUM
  bank at a time).
- Output tiles are evicted (bias fused, bf16 downcast) alternating
  between the Vector and Scalar engines, and stored on the ring
  opposite the evict engine, so no queue does evict+store
  back-to-back and the 8-bank burst at the end of bq0 drains in two
  parallel chains. The very last tile is split 256+128+128 so the
  final chain (evict -> 128-descriptor store issue -> HBM receipt)
  hangs off a narrow N=128 piece.
"""

import sys

if "/opt/trn_rl_repo" not in sys.path:
    sys.path.insert(0, "/opt/trn_rl_repo")

import ml_dtypes
import numpy as np

# concourse's trace path imports antenv.axon_hooks, which this image lacks.
# Register a working NTFF-profile hook (via the axon PJRT .so) so tracing
# works when requested, degrading to no-op if anything is missing.
try:
    import antenv.axon_hooks  # noqa: F401
except ImportError:
    import types as _types

    def _make_hook():
        try:
            from trn_agent_boot.trn_boot import _ntff_profile_via_ctypes

            return _ntff_profile_via_ctypes("/opt/axon/libaxon_pjrt.so")
        except Exception:
            return None

    _hooks = _types.ModuleType("antenv.axon_hooks")
    _hooks.get_axon_ntff_profile_hook = _make_hook
    _hooks.set_axon_ntff_profile_hook = lambda h: None
    sys.modules["antenv.axon_hooks"] = _hooks

BATCH = 16384
IN_F = 1024
OUT_F = 1024
NCORES = 8
P = 128
KO = IN_F // P  # 8 contraction tiles
MO = OUT_F // P  # 8 output-feature tiles
BS = BATCH // NCORES  # 2048 rows per core
FD = 512  # matmul moving free dim (one PSUM bank of fp32)
NB = BS // FD  # 4 batch chunks per core
# N=128 dummy matmuls bridging from body start (~7.5 us) until the
# first (w0, x00) pair lands (~10.7-11.5 us). Sized to just reach the
# data with NO multi-us PE idle gap: a big gap resets the HAM
# clock-gate ramp and costs ~3-5 us of half-clock matmuls. Sub-us
# undershoot is tolerated (measured: a 0.85 us gap did not reset it).
N_WARM = 31

_cache = {}
LAST_RESULT = None


def _build():
    import concourse.mybir as mybir
    import concourse.tile as tile
    from concourse import bacc

    nc = bacc.Bacc(None, target_bir_lowering=False)
    # xT4[p, bq, ko, fd] = x[bq*FD + fd, ko*P + p]
    xT = nc.declare_dram_parameter(
        "xT", [P, NB, KO, FD], mybir.dt.bfloat16, isOutput=False
    )
    # w3[p, ko, mo, c] = W[mo*P + c, ko*P + p]  (ko-major: bq0 consumes
    # weights one ko chunk at a time). NOTE: w and x must stay SEPARATE
    # SBUF tiles -- packing them into one tile puts the PE's two SBUF
    # read ports (weights + moving operand) on the same sub-banks and
    # slows every matmul ~20%.
    w3 = nc.declare_dram_parameter(
        "w3", [P, KO, MO, P], mybir.dt.bfloat16, isOutput=False
    )
    bias = nc.declare_dram_parameter("bias", [P, MO], mybir.dt.float32, isOutput=False)
    # out4[p, bq, mo, fd] = y[bq*FD + fd, mo*P + p]
    out = nc.declare_dram_parameter(
        "out", [P, NB, MO, FD], mybir.dt.bfloat16, isOutput=True
    )

    with tile.TileContext(nc) as tc:
        with (
            tc.tile_pool(name="const", bufs=1) as cpool,
            tc.tile_pool(name="outp", bufs=3) as opool,
            tc.tile_pool(name="psum", bufs=8, space="PSUM") as ppool,
        ):
            x_sb = cpool.tile([P, NB, KO, FD], mybir.dt.bfloat16)
            w_sb = cpool.tile([P, KO, MO, P], mybir.dt.bfloat16)
            b_sb = cpool.tile([P, MO], mybir.dt.float32)
            wu_sb = cpool.tile([P, P], mybir.dt.bfloat16)
            # memset on GpSimd: it has no other body work, so the first
            # warm-up matmul isn't queued behind Vector's preamble.
            nc.gpsimd.memset(wu_sb[:], 0.0)
            # All inputs on ONE HWDGE ring (sync), in consumption order.
            # A single ring streams (w, x) ko-pairs at ~1.35 us cadence;
            # phase A consumes a chunk in 1.73 us (2.4 GHz warm) or more,
            # so the stream stays ahead with no stalls. Splitting w onto a
            # second ring was measured WORSE: cross-ring SDMA interleaving
            # slowed the w stream enough to stall phase A ~4 us waiting on
            # late w chunks, and the first pair landed no earlier. FIFO
            # order also guarantees arrival order == consumption order.
            # NOTE: splitting w0 into halves (to start the first matmuls
            # ~0.3 us earlier) was measured WORSE: the extra issue slot
            # delays later pair chunks, phase A's thin DMA margin
            # (~0.35 us/chunk at 2.4 GHz) flips into a multi-us stall,
            # and that stall re-throttles the PE clock (HAM MID window).
            for ko in range(KO):
                nc.sync.dma_start(w_sb[:, ko], w3[:, ko])
                nc.sync.dma_start(x_sb[:, 0, ko], xT[:, 0, ko])
            nc.sync.dma_start(b_sb[:], bias[:])
            # Bulk x in half-chunks: phase B consumes ko-ascending, so
            # landing x1[ko0-3] ~3 us earlier removes a ~1 us stall at
            # the phase A -> B boundary (measured: phase B's first MM
            # waited on the full-1-MiB x1 transfer).
            for bq in range(1, NB):
                h = KO // 2
                nc.sync.dma_start(x_sb[:, bq, 0:h], xT[:, bq, 0:h])
                nc.sync.dma_start(x_sb[:, bq, h:KO], xT[:, bq, h:KO])

            # PE HAM warm-up: keep the PE continuously busy from body
            # start until the first input chunks land (~3 us), so the
            # clock-gate window starts counting as early as possible.
            wu_ps = ppool.tile([P, FD], mybir.dt.float32, tag="ps")
            for _ in range(N_WARM):
                nc.tensor.matmul(
                    wu_ps[:, :P], wu_sb[:], wu_sb[:], start=True, stop=True
                )

            # bq0: contraction-outer over all 8 PSUM banks, consuming
            # (w, x) ko-chunks in DMA arrival order.
            ps0 = [
                ppool.tile([P, FD], mybir.dt.float32, tag="ps", name=f"ps0_{mo}")
                for mo in range(MO)
            ]
            o_sb = opool.tile([P, MO, FD], mybir.dt.bfloat16)
            for ko in range(KO):
                for mo in range(MO):
                    nc.tensor.matmul(
                        ps0[mo][:],
                        w_sb[:, ko, mo],
                        x_sb[:, 0, ko],
                        start=(ko == 0),
                        stop=(ko == KO - 1),
                    )
            # Evictions alternate Vector/Scalar so the 8-bank burst at the
            # end of bq0 drains in two parallel ~2.4 us chains instead of
            # one ~5.5 us chain (whose backlog stalled later bank reuse).
            for mo in range(MO):
                if mo % 2 == 0:
                    nc.vector.tensor_scalar_add(
                        o_sb[:, mo], ps0[mo][:], b_sb[:, mo : mo + 1]
                    )
                else:
                    nc.scalar.activation(
                        o_sb[:, mo],
                        ps0[mo][:],
                        mybir.ActivationFunctionType.Identity,
                        bias=b_sb[:, mo : mo + 1],
                    )
                # Stores alternate rings, opposite to the evict engine,
                # so neither queue does evict+store back-to-back.
                st = nc.scalar if mo % 2 == 0 else nc.sync
                st.dma_start(out[:, 0, mo], o_sb[:, mo])

            # bq1..3: output-tile-outer, one PSUM bank at a time;
            # evict + store each tile as soon as it completes.
            for bq in range(1, NB):
                o_sb = opool.tile([P, MO, FD], mybir.dt.bfloat16)
                for mo in range(MO):
                    if bq == NB - 1 and mo == MO - 1:
                        # Very last tile: split 256+128+128 so the final
                        # dependency chain (evict -> 128-descriptor store
                        # issue -> HBM receipt) hangs off a narrow N=128
                        # piece. Evicts and stores alternate engines/rings
                        # so the three chains overlap.
                        pieces = [(0, 256), (256, 128), (384, 128)]
                        ev_eng = ["scalar", "vector", "vector"]
                        st_eng = [nc.sync, nc.scalar, nc.sync]
                        for h, (off, width) in enumerate(pieces):
                            hs = slice(off, off + width)
                            ps = ppool.tile(
                                [P, FD], mybir.dt.float32, tag="ps", name=f"ps_l{h}"
                            )
                            for ko in range(KO):
                                nc.tensor.matmul(
                                    ps[:, :width],
                                    w_sb[:, ko, mo],
                                    x_sb[:, bq, ko, hs],
                                    start=(ko == 0),
                                    stop=(ko == KO - 1),
                                )
                            if ev_eng[h] == "scalar":
                                nc.scalar.activation(
                                    o_sb[:, mo, hs],
                                    ps[:, :width],
                                    mybir.ActivationFunctionType.Identity,
                                    bias=b_sb[:, mo : mo + 1],
                                )
                            else:
                                nc.vector.tensor_scalar_add(
                                    o_sb[:, mo, hs], ps[:, :width], b_sb[:, mo : mo + 1]
                                )
                            st_eng[h].dma_start(out[:, bq, mo, hs], o_sb[:, mo, hs])
                    else:
                        ps = ppool.tile([P, FD], mybir.dt.float32, tag="ps")
                        for ko in range(KO):
                            nc.tensor.matmul(
                                ps[:],
                                w_sb[:, ko, mo],
                                x_sb[:, bq, ko],
                                start=(ko == 0),
                                stop=(ko == KO - 1),
                            )
                        if mo % 2 == 0:
                            nc.vector.tensor_scalar_add(
                                o_sb[:, mo], ps[:], b_sb[:, mo : mo + 1]
                            )
                        else:
                            nc.scalar.activation(
                                o_sb[:, mo],
                                ps[:],
                                mybir.ActivationFunctionType.Identity,
                                bias=b_sb[:, mo : mo + 1],
                            )
                        st = nc.scalar if mo % 2 == 0 else nc.sync
                        st.dma_start(out[:, bq, mo], o_sb[:, mo])

    nc.compile()
    return nc


def kernel(x, weight, bias):
    global LAST_RESULT
    from concourse.bass_utils import run_bass_kernel_spmd

    if "nc" not in _cache:
        _cache["nc"] = _build()
    nc = _cache["nc"]

    x = np.asarray(x, dtype=np.float32)
    weight = np.asarray(weight, dtype=np.float32)
    bias = np.asarray(bias, dtype=np.float32)

    bf16 = ml_dtypes.bfloat16
    # w3[p, ko, mo, c] = W[mo*P + c, ko*P + p]
    wb = weight.astype(bf16).reshape(MO, P, KO, P)  # [mo, c, ko, p]
    w3 = np.ascontiguousarray(wb.transpose(3, 2, 0, 1))  # [p, ko, mo, c]
    # bias laid out [P, MO]: b[p, mo] = bias[mo*P + p]
    b_t = np.ascontiguousarray(bias.astype(np.float32).reshape(MO, P).T)

    in_maps = []
    for c in range(NCORES):
        xs = x[c * BS : (c + 1) * BS].astype(bf16)
        # xT4[p, bq, ko, fd] = x[bq*FD + fd, ko*P + p]
        xr = xs.reshape(NB, FD, KO, P)  # [bq, fd, ko, p]
        xT = np.ascontiguousarray(xr.transpose(3, 0, 2, 1))  # [p, bq, ko, fd]
        in_maps.append({"xT": xT, "w3": w3, "bias": b_t})

    res = run_bass_kernel_spmd(nc, in_maps, list(range(NCORES)))
    LAST_RESULT = res

    y = np.empty((BATCH, OUT_F), dtype=np.float32)
    for c in range(NCORES):
        o = res.results[c]["out"]  # [p, bq, mo, fd] bf16
        y[c * BS : (c + 1) * BS] = (
            o.astype(np.float32).transpose(1, 3, 2, 0).reshape(BS, OUT_F)
        )
    return y

